# revision 6
# baseline (speedup 1.0000x reference)
"""MultiHeadAttention (B=2, S=2048, D=1024, H=16, causal) on 8 trn2 NeuronCores.

Sharding: tensor-parallel over heads (2 heads/core) for QKV projections and
attention; AllToAll re-shards context rows so the output projection is
data-parallel over the 4096 (batch*seq) rows (512 rows/core); bias added on
device. Host only slices/transposes/casts inputs and concatenates outputs.

Math notes:
  - torch-Linear semantics: q = x @ Wq.T etc. Host passes transposed weight
    shards so all matmuls contract over the SBUF partition dim.
  - softmax without max-subtraction (scores*inv_scale is O(1), exp is safe);
    denominator comes from a ones-column appended to V in the attn@V matmul.
  - reference quirk preserved: scale = 1/(D**0.25).
"""

import math
import sys
import types

import numpy as np
import ml_dtypes

N_CORES = 8
B, S, D = 2, 2048, 1024
H = 16
HEAD = 64
ROWS = B * S               # 4096
ROWS_PER_CORE = ROWS // N_CORES  # 512
INV_SCALE = 1.0 / (D ** 0.25)
QT = 512                   # query tile (free dim)
KT = 128                   # key tile (partition dim)
N_QT = S // QT             # 4 query tiles per batch
N_KT_B = S // KT           # 16 key tiles per batch

BF16 = ml_dtypes.bfloat16

_compiled = None


def _install_axon_profile_shim():
    """Provide antenv.axon_hooks (missing from this image) so trace=True works,
    and neuter the artifact upload (no bucket access in-container)."""
    if "antenv.axon_hooks" not in sys.modules:
        mod = types.ModuleType("antenv.axon_hooks")
        mod._hook = None
        mod.set_axon_ntff_profile_hook = lambda h: setattr(mod, "_hook", h)
        mod.get_axon_ntff_profile_hook = lambda: mod._hook
        sys.modules["antenv.axon_hooks"] = mod
        try:
            import antenv
            antenv.axon_hooks = mod
        except ImportError:
            pass
    mod = sys.modules["antenv.axon_hooks"]
    if mod._hook is None:
        try:
            from trn_agent_boot.trn_boot import _ntff_profile_via_ctypes
            mod.set_axon_ntff_profile_hook(
                _ntff_profile_via_ctypes("/opt/axon/libaxon_pjrt.so"))
        except Exception:
            pass
    try:
        import concourse.bass_utils as bu
        bu.upload_artifacts = lambda tmpdir: tmpdir
    except Exception:
        pass


def _split_excess_waits(nc, max_waits=1):
    """walrus in this container only encodes one sem-wait per instruction;
    hoist extras onto InstEventSemaphore instructions inserted just before."""
    import concourse.mybir as mybir
    n = 0
    for fn in nc.m.functions:
        for bb in fn.blocks:
            out = []
            for inst in bb.instructions:
                si = inst.sync_info
                if si is not None and si.on_wait and len(si.on_wait) > max_waits:
                    waits = list(si.on_wait)
                    excess, keep = waits[:-max_waits], waits[-max_waits:]
                    for i in range(0, len(excess), max_waits):
                        ev = mybir.InstEventSemaphore(
                            name=f"{inst.name}-wsplit{n}",
                            engine=inst.engine,
                            ins=[], outs=[],
                            sync_info=mybir.SyncInfo(
                                on_wait=list(excess[i:i + max_waits]),
                                on_update=[]),
                        )
                        n += 1
                        out.append(ev)
                    si.on_wait = keep
                out.append(inst)
            bb.instructions = out
    return n


def _build_program(debug=False):
    import concourse.bass as bass
    import concourse.mybir as mybir
    import concourse.tile as tile
    from concourse.bass import ts

    f32 = mybir.dt.float32
    bf16 = mybir.dt.bfloat16
    Exp = mybir.ActivationFunctionType.Exp

    nc = bass.Bass(num_devices=N_CORES)
    dbg = {}
    if debug:
        dbg["qT"] = nc.dram_tensor("dbg_qT", [128, ROWS], bf16,
                                   kind="ExternalOutput")
        dbg["kT"] = nc.dram_tensor("dbg_kT", [128, ROWS], bf16,
                                   kind="ExternalOutput")
        dbg["v"] = nc.dram_tensor("dbg_v", [128, 32, 2, HEAD + 1], bf16,
                                  kind="ExternalOutput")
        dbg["ctx0"] = nc.dram_tensor("dbg_ctx0", [64, ROWS], bf16,
                                     kind="ExternalOutput")
        dbg["ctx1"] = nc.dram_tensor("dbg_ctx1", [64, ROWS], bf16,
                                     kind="ExternalOutput")
        dbg["a2a"] = nc.dram_tensor("dbg_a2a", [128, 8, QT], bf16,
                                    kind="ExternalOutput")

    xT = nc.dram_tensor("xT", [D, ROWS], bf16, kind="ExternalInput")
    wqT = nc.dram_tensor("wqT", [D, 128], bf16, kind="ExternalInput")
    wkT = nc.dram_tensor("wkT", [D, 128], bf16, kind="ExternalInput")
    wvT = nc.dram_tensor("wvT", [D, 128], bf16, kind="ExternalInput")
    woT = nc.dram_tensor("woT", [D, D], bf16, kind="ExternalInput")
    bo = nc.dram_tensor("bo", [D], f32, kind="ExternalInput")
    maskt = nc.dram_tensor("maskt", [4, 128, QT], bf16, kind="ExternalInput")
    out_rows = nc.dram_tensor("out_rows", [ROWS_PER_CORE, D], f32,
                              kind="ExternalOutput")

    with tile.TileContext(nc) as tc:
        with (
            tc.tile_pool(name="persist", bufs=1) as persist,
            tc.tile_pool(name="cp", bufs=4) as cp,
            tc.tile_pool(name="attn", bufs=4) as attn_pool,
            tc.tile_pool(name="nrm", bufs=4) as nrm_pool,
            tc.tile_pool(name="ps_work", bufs=2, space="PSUM") as ps_work,
            tc.tile_pool(name="ps_scores", bufs=2, space="PSUM") as ps_scores,
            tc.tile_pool(name="ps_bc", bufs=2, space="PSUM") as ps_bc,
            tc.tile_pool(name="dram", bufs=1, space="DRAM") as dram,
        ):
            # ---- persistent SBUF state ----
            xT_sb = persist.tile([128, 8, ROWS], bf16)        # 64 KB/part
            wq_sb = persist.tile([128, 8, 128], bf16)
            wk_sb = persist.tile([128, 8, 128], bf16)
            wv_sb = persist.tile([128, 8, 128], bf16)
            woT_sb = persist.tile([128, 8, D], bf16)          # 16 KB/part
            qT_sb = persist.tile([128, ROWS], bf16)           # 8 KB/part
            kT_sb = persist.tile([128, ROWS], bf16)
            v_sb = persist.tile([128, 32, 2, HEAD + 1], bf16)  # rows-tiled V+ones
            ctx_sb = [persist.tile([64, ROWS], bf16, tag=f"ctx{h}",
                                   name=f"ctx{h}")
                      for h in range(2)]
            mask_sb = persist.tile([128, 4, QT], bf16)
            ones_sb = persist.tile([128, HEAD], f32)
            bo_sb = persist.tile([128, D], f32)
            a2a_sb = persist.tile([128, 8, QT], bf16)

            for kt in range(8):
                nc.sync.dma_start(xT_sb[:, kt, :], xT[ts(kt, 128), :])
            nc.sync.dma_start(wq_sb[:], wqT.rearrange("(t p) m -> p t m", p=128))
            nc.sync.dma_start(wk_sb[:], wkT.rearrange("(t p) m -> p t m", p=128))
            nc.sync.dma_start(wv_sb[:], wvT.rearrange("(t p) m -> p t m", p=128))
            nc.sync.dma_start(woT_sb[:], woT.rearrange("(t p) o -> p t o", p=128))
            nc.sync.dma_start(mask_sb[:], maskt.rearrange("o p r -> p o r"))
            nc.sync.dma_start(
                bo_sb[:], bass.AP(tensor=bo, offset=0,
                                  ap=[[0, 128], [1, D]]))
            nc.vector.memset(ones_sb[:], 1.0)
            nc.vector.memset(v_sb[:, :, :, HEAD], 1.0)

            # ---- Q/K projections: qT/kT [128 feat, 4096 rows] ----
            for w_sb, dst in ((wq_sb, qT_sb), (wk_sb, kT_sb)):
                for rt in range(8):
                    ps = ps_work.tile([128, 512], f32, tag="work")
                    for kt in range(8):
                        nc.tensor.matmul(ps, w_sb[:, kt, :],
                                         xT_sb[:, kt, ts(rt, 512)],
                                         start=(kt == 0), stop=(kt == 7))
                    nc.any.tensor_copy(dst[:, ts(rt, 512)], ps)

            # ---- V projection: natural layout [rows, feat], per 128-row tile ----
            for rt in range(32):
                ps = ps_work.tile([128, 512], f32, tag="work")
                for kt in range(8):
                    nc.tensor.matmul(ps[:, 0:128],
                                     xT_sb[:, kt, ts(rt, 128)],
                                     wv_sb[:, kt, :],
                                     start=(kt == 0), stop=(kt == 7))
                nc.any.tensor_copy(
                    v_sb[:, rt, :, 0:HEAD],
                    ps[:, 0:128].rearrange("p (h f) -> p h f", h=2))

            # ---- attention per (batch, head, query-tile) ----
            for b in range(B):
                for h in range(2):
                    hs = slice(h * HEAD, (h + 1) * HEAD)
                    for qt in range(N_QT):
                        q0 = b * S + qt * QT
                        n_k = 4 * qt + 4
                        ps_av = ps_work.tile([HEAD + 1, QT], f32, tag="work")
                        for pair in range(n_k // 2):
                            ps_s = ps_scores.tile([128, 2, QT], f32, tag="sc")
                            at = attn_pool.tile([128, 2, QT], bf16, tag="at")
                            jks = (2 * pair, 2 * pair + 1)
                            for i, jk in enumerate(jks):
                                k0 = b * S + jk * KT
                                nc.tensor.matmul(
                                    ps_s[:, i, :],
                                    kT_sb[hs, k0:k0 + KT],
                                    qT_sb[hs, q0:q0 + QT],
                                    start=True, stop=True)
                            nc.scalar.activation(at[:], ps_s[:], Exp,
                                                 scale=INV_SCALE)
                            for i, jk in enumerate(jks):
                                o = jk - 4 * qt
                                if o >= 0:
                                    nc.vector.tensor_mul(at[:, i, :],
                                                         at[:, i, :],
                                                         mask_sb[:, o, :])
                                nc.tensor.matmul(
                                    ps_av[:],
                                    v_sb[:, b * 16 + jk, h, :],
                                    at[:, i, :],
                                    start=(jk == 0), stop=(jk == n_k - 1))
                        # normalize: ctx[f, i] = av[f, i] / av[HEAD, i]
                        rec = nrm_pool.tile([128, QT], f32, tag="rec")
                        nc.vector.reciprocal(rec[HEAD:HEAD + 1, :],
                                             ps_av[HEAD:HEAD + 1, :])
                        ps_b = ps_bc.tile([HEAD, QT], f32, tag="bc")
                        nc.tensor.matmul(ps_b,
                                         ones_sb[HEAD:HEAD + 1, 0:HEAD],
                                         rec[HEAD:HEAD + 1, :],
                                         start=True, stop=True)
                        bc = nrm_pool.tile([HEAD, QT], f32, tag="bcs")
                        nc.any.tensor_copy(bc, ps_b)
                        nc.vector.tensor_mul(ctx_sb[h][:, q0:q0 + QT],
                                             ps_av[0:HEAD, :], bc[:])

            # ---- AllToAll: re-shard ctx from head-major to row-major ----
            a2a_in = dram.tile([8, 128, QT], bf16)
            a2a_out = dram.tile([8, 128, QT], bf16)
            nc.sync.dma_start(
                a2a_in[:, 0:64, :].rearrange("s p r -> p s r"),
                ctx_sb[0][:].rearrange("p (s r) -> p s r", s=8))
            nc.sync.dma_start(
                a2a_in[:, 64:128, :].rearrange("s p r -> p s r"),
                ctx_sb[1][:].rearrange("p (s r) -> p s r", s=8))
            nc.gpsimd.collective_compute(
                "AllToAll", mybir.AluOpType.bypass,
                replica_groups=[list(range(N_CORES))],
                ins=[a2a_in[:].opt()], outs=[a2a_out[:].opt()])
            nc.sync.dma_start(a2a_sb[:],
                              a2a_out[:].rearrange("t p r -> p t r"))

            if debug:
                nc.sync.dma_start(dbg["qT"][:], qT_sb[:])
                nc.sync.dma_start(dbg["kT"][:], kT_sb[:])
                nc.sync.dma_start(dbg["v"][:], v_sb[:])
                nc.sync.dma_start(dbg["ctx0"][:], ctx_sb[0][:])
                nc.sync.dma_start(dbg["ctx1"][:], ctx_sb[1][:])
                nc.sync.dma_start(dbg["a2a"][:], a2a_sb[:])

            # ---- output projection + bias on this core's 512 rows ----
            for rt in range(4):
                for nh in range(2):
                    ps = ps_work.tile([128, 512], f32, tag="work")
                    for t in range(8):
                        nc.tensor.matmul(ps,
                                         a2a_sb[:, t, ts(rt, 128)],
                                         woT_sb[:, t, ts(nh, 512)],
                                         start=(t == 0), stop=(t == 7))
                    ob = cp.tile([128, 512], f32, tag="ob")
                    nc.vector.tensor_add(ob, ps, bo_sb[:, ts(nh, 512)])
                    nc.sync.dma_start(
                        out_rows[ts(rt, 128), ts(nh, 512)], ob)

    _split_excess_waits(nc)
    return nc


def _make_masks():
    # mask[o][p, r] = 1 if key row (128*o + p) <= query row r (within tile)
    o = np.arange(4)[:, None, None]
    p = np.arange(128)[None, :, None]
    r = np.arange(QT)[None, None, :]
    return ((128 * o + p) <= r).astype(BF16)


def _shard_inputs(x, Wq, Wk, Wv, Wo, bo):
    xT = np.ascontiguousarray(
        x.reshape(ROWS, D).T).astype(BF16)            # [D, 4096]
    woT = np.ascontiguousarray(Wo.T).astype(BF16)     # [D, D]
    masks = _make_masks()
    bo32 = np.ascontiguousarray(bo.astype(np.float32))
    maps = []
    for c in range(N_CORES):
        rs = slice(c * 128, (c + 1) * 128)
        maps.append({
            "xT": xT,
            "wqT": np.ascontiguousarray(Wq[rs].T).astype(BF16),
            "wkT": np.ascontiguousarray(Wk[rs].T).astype(BF16),
            "wvT": np.ascontiguousarray(Wv[rs].T).astype(BF16),
            "woT": woT,
            "bo": bo32,
            "maskt": masks,
        })
    return maps


def kernel(x, Wq, Wk, Wv, Wo, bo, trace=False):
    global _compiled
    _install_axon_profile_shim()
    from concourse.bass_utils import run_bass_kernel_spmd

    x = np.asarray(x, dtype=np.float32)
    Wq = np.asarray(Wq, dtype=np.float32)
    Wk = np.asarray(Wk, dtype=np.float32)
    Wv = np.asarray(Wv, dtype=np.float32)
    Wo = np.asarray(Wo, dtype=np.float32)
    bo = np.asarray(bo, dtype=np.float32)

    if _compiled is None:
        _compiled = _build_program()
    nc = _compiled

    in_maps = _shard_inputs(x, Wq, Wk, Wv, Wo, bo)
    res = run_bass_kernel_spmd(nc, in_maps, core_ids=list(range(N_CORES)),
                               trace=trace)
    out = np.concatenate(
        [res.results[c]["out_rows"] for c in range(N_CORES)], axis=0)
    out = out.reshape(B, S, D).astype(np.float32)
    if trace:
        kernel.last_exec_time_ns = res.exec_time_ns
        kernel.last_results = res
    return out


# revision 11
# speedup vs baseline: 1.2525x; 1.2525x over previous
"""MultiHeadAttention (B=2, S=2048, D=1024, H=16, causal) on 8 trn2 NeuronCores.

Sharding: tensor-parallel over heads (2 heads/core) for QKV projections and
attention; AllToAll re-shards context rows so the output projection is
data-parallel over the 4096 (batch*seq) rows (512 rows/core); bias added on
device. Host only slices/transposes/casts inputs and concatenates outputs.

Math notes:
  - torch-Linear semantics: q = x @ Wq.T etc. Host passes transposed weight
    shards so all matmuls contract over the SBUF partition dim.
  - softmax without max-subtraction (scores*inv_scale is O(1), exp is safe);
    denominator comes from a ones-column appended to V in the attn@V matmul.
  - normalization is deferred and batched: per-unit denominators are copied
    out, reciprocal'd in one 16-lane op, broadcast across partitions with a
    selector matmul, and multiplied in at the end (keeps PE unstalled).
  - reference quirk preserved: scale = 1/(D**0.25).
"""

import math
import sys
import types

import numpy as np
import ml_dtypes

N_CORES = 8
B, S, D = 2, 2048, 1024
H = 16
HEAD = 64
ROWS = B * S               # 4096
ROWS_PER_CORE = ROWS // N_CORES  # 512
INV_SCALE = 1.0 / (D ** 0.25)
QT = 512                   # query tile (free dim)
KT = 128                   # key tile (partition dim)
N_QT = S // QT             # 4 query tiles per batch
N_UNITS = B * N_QT * 2     # (b, qt, h) units

BF16 = ml_dtypes.bfloat16

_compiled = None


def _install_axon_profile_shim():
    """Provide antenv.axon_hooks (missing from this image) so trace=True works,
    and neuter the artifact upload (no bucket access in-container)."""
    if "antenv.axon_hooks" not in sys.modules:
        mod = types.ModuleType("antenv.axon_hooks")
        mod._hook = None
        mod.set_axon_ntff_profile_hook = lambda h: setattr(mod, "_hook", h)
        mod.get_axon_ntff_profile_hook = lambda: mod._hook
        sys.modules["antenv.axon_hooks"] = mod
        try:
            import antenv
            antenv.axon_hooks = mod
        except ImportError:
            pass
    mod = sys.modules["antenv.axon_hooks"]
    if mod._hook is None:
        try:
            from trn_agent_boot.trn_boot import _ntff_profile_via_ctypes
            mod.set_axon_ntff_profile_hook(
                _ntff_profile_via_ctypes("/opt/axon/libaxon_pjrt.so"))
        except Exception:
            pass
    try:
        import concourse.bass_utils as bu
        bu.upload_artifacts = lambda tmpdir: tmpdir
    except Exception:
        pass


def _split_excess_waits(nc, max_waits=1):
    """walrus in this container only encodes one sem-wait per instruction;
    hoist extras onto InstEventSemaphore instructions inserted just before."""
    import concourse.mybir as mybir
    n = 0
    for fn in nc.m.functions:
        for bb in fn.blocks:
            out = []
            for inst in bb.instructions:
                si = inst.sync_info
                if si is not None and si.on_wait and len(si.on_wait) > max_waits:
                    waits = list(si.on_wait)
                    excess, keep = waits[:-max_waits], waits[-max_waits:]
                    for i in range(0, len(excess), max_waits):
                        ev = mybir.InstEventSemaphore(
                            name=f"{inst.name}-wsplit{n}",
                            engine=inst.engine,
                            ins=[], outs=[],
                            sync_info=mybir.SyncInfo(
                                on_wait=list(excess[i:i + max_waits]),
                                on_update=[]),
                        )
                        n += 1
                        out.append(ev)
                    si.on_wait = keep
                out.append(inst)
            bb.instructions = out
    return n


def _build_program(debug=False):
    import concourse.bass as bass
    import concourse.mybir as mybir
    import concourse.tile as tile
    from concourse.bass import ts

    f32 = mybir.dt.float32
    f32r = mybir.dt.float32r
    bf16 = mybir.dt.bfloat16
    Exp = mybir.ActivationFunctionType.Exp

    nc = bass.Bass(num_devices=N_CORES)
    dbg = {}
    if debug:
        dbg["qT"] = nc.dram_tensor("dbg_qT", [128, ROWS], bf16,
                                   kind="ExternalOutput")
        dbg["kT"] = nc.dram_tensor("dbg_kT", [128, ROWS], bf16,
                                   kind="ExternalOutput")
        dbg["v"] = nc.dram_tensor("dbg_v", [128, 32, 2, HEAD + 1], bf16,
                                  kind="ExternalOutput")  # filled per tile
        dbg["ctx0"] = nc.dram_tensor("dbg_ctx0", [64, ROWS], bf16,
                                     kind="ExternalOutput")
        dbg["ctx1"] = nc.dram_tensor("dbg_ctx1", [64, ROWS], bf16,
                                     kind="ExternalOutput")
        dbg["a2a"] = nc.dram_tensor("dbg_a2a", [128, 8, QT], bf16,
                                    kind="ExternalOutput")

    xT = nc.dram_tensor("xT", [D, ROWS], bf16, kind="ExternalInput")
    wqT = nc.dram_tensor("wqT", [D, 128], bf16, kind="ExternalInput")
    wkT = nc.dram_tensor("wkT", [D, 128], bf16, kind="ExternalInput")
    wvT = nc.dram_tensor("wvT", [D, 128], bf16, kind="ExternalInput")
    woT = nc.dram_tensor("woT", [D, D], bf16, kind="ExternalInput")
    bo = nc.dram_tensor("bo", [D], f32, kind="ExternalInput")
    masksq = nc.dram_tensor("masksq", [128, 128], bf16, kind="ExternalInput")
    sel = nc.dram_tensor("sel", [16, 16 * HEAD], f32r, kind="ExternalInput")
    out_rows = nc.dram_tensor("out_rows", [ROWS_PER_CORE, D], f32,
                              kind="ExternalOutput")

    with tile.TileContext(nc) as tc:
        with (
            tc.tile_pool(name="persist", bufs=1) as persist,
            tc.tile_pool(name="cp", bufs=4) as cp,
            tc.tile_pool(name="attn", bufs=4) as attn_pool,
            tc.tile_pool(name="nrm", bufs=4) as nrm_pool,
            tc.tile_pool(name="ps_work", bufs=3, space="PSUM") as ps_work,
            tc.tile_pool(name="ps_scores", bufs=2, space="PSUM") as ps_scores,
            tc.tile_pool(name="dram", bufs=1, space="DRAM") as dram,
        ):
            # ---- persistent SBUF state ----
            xT_sb = persist.tile([128, 8, ROWS], bf16)        # 64 KB/part
            wq_sb = persist.tile([128, 8, 128], bf16)
            wk_sb = persist.tile([128, 8, 128], bf16)
            wv_sb = persist.tile([128, 8, 128], bf16)
            woT_sb = persist.tile([128, 8, D], bf16)          # 16 KB/part
            qT_sb = persist.tile([128, ROWS], bf16)           # 8 KB/part
            kT_sb = persist.tile([128, ROWS], bf16)
            vT_sb = persist.tile([128, ROWS], bf16)
            # one tile per (rowtile, head): xbar-transpose needs offset-0
            # contiguous dst; col 64 is the ones column for the denominator
            v_tiles = [[persist.tile([128, HEAD + 1], bf16, tag=f"v{rt}_{h}",
                                     name=f"v{rt}_{h}")
                        for h in range(2)] for rt in range(32)]
            ctx_sb = [persist.tile([64, ROWS], bf16, tag=f"ctx{h}",
                                   name=f"ctx{h}")
                      for h in range(2)]
            mask_sb = persist.tile([128, 128], bf16)
            sel_sb = persist.tile([16, 16 * HEAD], f32r)
            den_all = persist.tile([16, QT], f32)
            den_rec = persist.tile([16, QT], f32r)
            bo_sb = persist.tile([128, D], f32)
            a2a_sb = persist.tile([128, 8, QT], bf16)

            for kt in range(8):
                nc.sync.dma_start(xT_sb[:, kt, :], xT[ts(kt, 128), :])
            nc.sync.dma_start(wq_sb[:], wqT.rearrange("(t p) m -> p t m", p=128))
            nc.sync.dma_start(wk_sb[:], wkT.rearrange("(t p) m -> p t m", p=128))
            nc.sync.dma_start(wv_sb[:], wvT.rearrange("(t p) m -> p t m", p=128))
            nc.sync.dma_start(woT_sb[:], woT.rearrange("(t p) o -> p t o", p=128))
            nc.sync.dma_start(mask_sb[:], masksq[:])
            nc.sync.dma_start(sel_sb[:], sel[:])
            nc.sync.dma_start(
                bo_sb[:], bass.AP(tensor=bo, offset=0,
                                  ap=[[0, 128], [1, D]]))
            for rt in range(32):
                for h in range(2):
                    nc.gpsimd.memset(v_tiles[rt][h][:, HEAD:HEAD + 1], 1.0)

            # ---- Q/K/V projections: qT/kT/vT [128 feat, 4096 rows] ----
            for w_sb, dst in ((wq_sb, qT_sb), (wk_sb, kT_sb), (wv_sb, vT_sb)):
                for rt in range(8):
                    ps = ps_work.tile([128, 512], f32, tag="work")
                    for kt in range(8):
                        nc.tensor.matmul(ps, w_sb[:, kt, :],
                                         xT_sb[:, kt, ts(rt, 512)],
                                         start=(kt == 0), stop=(kt == 7))
                    nc.any.tensor_copy(dst[:, ts(rt, 512)], ps)

            # vT -> v (natural layout) via xbar DMA transpose, per head/rowtile
            for rt in range(32):
                for h in range(2):
                    nc.sync.dma_start_transpose(
                        v_tiles[rt][h][:, 0:HEAD],
                        vT_sb[h * HEAD:(h + 1) * HEAD, ts(rt, 128)])

            # ---- attention per (batch, query-tile), both heads fused ----
            uidx = 0
            unit_ctx_slices = []
            for b in range(B):
                for qt in range(N_QT):
                    q0 = b * S + qt * QT
                    n_k = 4 * qt + 4
                    ps_av = [ps_work.tile([HEAD + 1, QT], f32, tag="work",
                                          name=f"av{b}_{qt}_{h}")
                             for h in range(2)]
                    for jk in range(n_k):
                        o = jk - 4 * qt       # >=0 on the diagonal band
                        c0 = max(o, 0) * 128  # first live query column
                        w = QT - c0
                        k0 = b * S + jk * KT
                        ps_s = ps_scores.tile([128, 2, QT], f32, tag="sc")
                        at = attn_pool.tile([128, 2, QT], bf16, tag="at")
                        for h in range(2):
                            hs = slice(h * HEAD, (h + 1) * HEAD)
                            nc.tensor.matmul(
                                ps_s[:, h, c0:QT],
                                kT_sb[hs, k0:k0 + KT],
                                qT_sb[hs, q0 + c0:q0 + QT],
                                start=True, stop=True)
                        nc.scalar.activation(at[:, :, c0:QT], ps_s[:, :, c0:QT],
                                             Exp, scale=INV_SCALE)
                        if o >= 0:
                            # partial causal sub-block: cols [c0, c0+128)
                            nc.vector.tensor_mul(
                                at[:, :, c0:c0 + 128],
                                at[:, :, c0:c0 + 128],
                                mask_sb[:, None, :].to_broadcast([128, 2, 128]))
                        for h in range(2):
                            nc.tensor.matmul(
                                ps_av[h][:, c0:QT],
                                v_tiles[b * 16 + jk][h][:],
                                at[:, h, c0:QT],
                                start=(jk == 0), stop=(jk == n_k - 1))
                    for h in range(2):
                        # stash unnormalized ctx + denominator row
                        nc.vector.tensor_copy(ctx_sb[h][:, q0:q0 + QT],
                                              ps_av[h][0:HEAD, :])
                        dtmp = nrm_pool.tile([HEAD + 1, QT], f32, tag="dtmp")
                        nc.vector.tensor_copy(dtmp[HEAD:HEAD + 1, :],
                                              ps_av[h][HEAD:HEAD + 1, :])
                        nc.sync.dma_start(den_all[uidx:uidx + 1, :],
                                          dtmp[HEAD:HEAD + 1, :])
                        unit_ctx_slices.append((h, q0))
                        uidx += 1

            # ---- batched softmax normalization ----
            with nc.allow_low_precision(
                    reason="softmax denominators: f32r keeps ~19 mantissa "
                           "bits, ample for a 1/x broadcast"):
                nc.vector.reciprocal(den_rec[:], den_all[:])
            for u, (h, q0) in enumerate(unit_ctx_slices):
                ps_b = ps_scores.tile([HEAD, QT], f32, tag="sc")
                nc.tensor.matmul(ps_b,
                                 sel_sb[:, ts(u, HEAD)],
                                 den_rec[:],
                                 start=True, stop=True)
                nc.vector.tensor_mul(ctx_sb[h][:, q0:q0 + QT],
                                     ctx_sb[h][:, q0:q0 + QT],
                                     ps_b[:])

            # ---- AllToAll: re-shard ctx from head-major to row-major ----
            a2a_in = dram.tile([8, 128, QT], bf16)
            a2a_out = dram.tile([8, 128, QT], bf16)
            nc.sync.dma_start(
                a2a_in[:, 0:64, :].rearrange("s p r -> p s r"),
                ctx_sb[0][:].rearrange("p (s r) -> p s r", s=8))
            nc.sync.dma_start(
                a2a_in[:, 64:128, :].rearrange("s p r -> p s r"),
                ctx_sb[1][:].rearrange("p (s r) -> p s r", s=8))
            nc.gpsimd.collective_compute(
                "AllToAll", mybir.AluOpType.bypass,
                replica_groups=[list(range(N_CORES))],
                ins=[a2a_in[:].opt()], outs=[a2a_out[:].opt()])
            nc.sync.dma_start(a2a_sb[:],
                              a2a_out[:].rearrange("t p r -> p t r"))

            if debug:
                nc.sync.dma_start(dbg["qT"][:], qT_sb[:])
                nc.sync.dma_start(dbg["kT"][:], kT_sb[:])
                for rt in range(32):
                    for h in range(2):
                        nc.sync.dma_start(dbg["v"][:, rt, h, :],
                                          v_tiles[rt][h][:])
                nc.sync.dma_start(dbg["ctx0"][:], ctx_sb[0][:])
                nc.sync.dma_start(dbg["ctx1"][:], ctx_sb[1][:])
                nc.sync.dma_start(dbg["a2a"][:], a2a_sb[:])

            # ---- output projection + bias on this core's 512 rows ----
            for rt in range(4):
                for nh in range(2):
                    ps = ps_work.tile([128, 512], f32, tag="work")
                    for t in range(8):
                        nc.tensor.matmul(ps,
                                         a2a_sb[:, t, ts(rt, 128)],
                                         woT_sb[:, t, ts(nh, 512)],
                                         start=(t == 0), stop=(t == 7))
                    ob = cp.tile([128, 512], f32, tag="ob")
                    nc.vector.tensor_add(ob, ps, bo_sb[:, ts(nh, 512)])
                    nc.sync.dma_start(
                        out_rows[ts(rt, 128), ts(nh, 512)], ob)

    _split_excess_waits(nc)
    return nc


def _make_masksq():
    p = np.arange(128)[:, None]
    r = np.arange(128)[None, :]
    return (p <= r).astype(BF16)


def _make_sel():
    # sel[k, u*64+m] = 1 if k == u : broadcasts den_rec row u over 64 partitions
    s = np.zeros((16, 16 * HEAD), np.float32)
    for u in range(16):
        s[u, u * HEAD:(u + 1) * HEAD] = 1.0
    return s


def _shard_inputs(x, Wq, Wk, Wv, Wo, bo):
    xT = np.ascontiguousarray(
        x.reshape(ROWS, D).T).astype(BF16)            # [D, 4096]
    woT = np.ascontiguousarray(Wo.T).astype(BF16)     # [D, D]
    masksq = _make_masksq()
    sel = _make_sel()
    bo32 = np.ascontiguousarray(bo.astype(np.float32))
    maps = []
    for c in range(N_CORES):
        rs = slice(c * 128, (c + 1) * 128)
        maps.append({
            "xT": xT,
            "wqT": np.ascontiguousarray(Wq[rs].T).astype(BF16),
            "wkT": np.ascontiguousarray(Wk[rs].T).astype(BF16),
            "wvT": np.ascontiguousarray(Wv[rs].T).astype(BF16),
            "woT": woT,
            "bo": bo32,
            "masksq": masksq,
            "sel": sel,
        })
    return maps


def kernel(x, Wq, Wk, Wv, Wo, bo, trace=False):
    global _compiled
    _install_axon_profile_shim()
    from concourse.bass_utils import run_bass_kernel_spmd

    x = np.asarray(x, dtype=np.float32)
    Wq = np.asarray(Wq, dtype=np.float32)
    Wk = np.asarray(Wk, dtype=np.float32)
    Wv = np.asarray(Wv, dtype=np.float32)
    Wo = np.asarray(Wo, dtype=np.float32)
    bo = np.asarray(bo, dtype=np.float32)

    if _compiled is None:
        _compiled = _build_program()
    nc = _compiled

    in_maps = _shard_inputs(x, Wq, Wk, Wv, Wo, bo)
    res = run_bass_kernel_spmd(nc, in_maps, core_ids=list(range(N_CORES)),
                               trace=trace)
    out = np.concatenate(
        [res.results[c]["out_rows"] for c in range(N_CORES)], axis=0)
    out = out.reshape(B, S, D).astype(np.float32)
    if trace:
        kernel.last_exec_time_ns = res.exec_time_ns
        kernel.last_results = res
    return out


# revision 12
# speedup vs baseline: 1.4093x; 1.1251x over previous
"""MultiHeadAttention (B=2, S=2048, D=1024, H=16, causal) on 8 trn2 NeuronCores.

Sharding: tensor-parallel over heads (2 heads/core) for QKV projections and
attention; two AllToAlls (one per batch) re-shard context rows so the output
projection is data-parallel over rows; bias added on device. Host only
slices/transposes/casts inputs and reassembles outputs.

Per-core output rows: global rows [c*256,(c+1)*256) (batch 0 part) and
[2048+c*256, 2048+(c+1)*256) (batch 1 part) — the per-batch A2A split lets
batch-0's collective and output projection overlap batch-1's attention.

Math notes:
  - torch-Linear semantics: q = x @ Wq.T etc. Host passes transposed weight
    shards so all matmuls contract over the SBUF partition dim.
  - softmax without max-subtraction (scores*inv_scale is O(1), exp is safe);
    denominator comes from a ones-column appended to V in the attn@V matmul.
  - normalization is deferred and batched per batch: denominators are copied
    out, reciprocal'd in one 8-lane op, broadcast across partitions with a
    selector matmul (f32r), and multiplied in (keeps PE unstalled).
  - reference quirk preserved: scale = 1/(D**0.25).
"""

import math
import sys
import types

import numpy as np
import ml_dtypes

N_CORES = 8
B, S, D = 2, 2048, 1024
H = 16
HEAD = 64
ROWS = B * S               # 4096
ROWS_PER_CORE = ROWS // N_CORES  # 512
INV_SCALE = 1.0 / (D ** 0.25)
QT = 512                   # query tile (free dim)
KT = 128                   # key tile (partition dim)
N_QT = S // QT             # 4 query tiles per batch
RH = 256                   # rows per core per batch-half

BF16 = ml_dtypes.bfloat16

_compiled = None


def _install_axon_profile_shim():
    """Provide antenv.axon_hooks (missing from this image) so trace=True works,
    and neuter the artifact upload (no bucket access in-container)."""
    if "antenv.axon_hooks" not in sys.modules:
        mod = types.ModuleType("antenv.axon_hooks")
        mod._hook = None
        mod.set_axon_ntff_profile_hook = lambda h: setattr(mod, "_hook", h)
        mod.get_axon_ntff_profile_hook = lambda: mod._hook
        sys.modules["antenv.axon_hooks"] = mod
        try:
            import antenv
            antenv.axon_hooks = mod
        except ImportError:
            pass
    mod = sys.modules["antenv.axon_hooks"]
    if mod._hook is None:
        try:
            from trn_agent_boot.trn_boot import _ntff_profile_via_ctypes
            mod.set_axon_ntff_profile_hook(
                _ntff_profile_via_ctypes("/opt/axon/libaxon_pjrt.so"))
        except Exception:
            pass
    try:
        import concourse.bass_utils as bu
        bu.upload_artifacts = lambda tmpdir: tmpdir
    except Exception:
        pass


def _split_excess_waits(nc, max_waits=1):
    """walrus in this container only encodes one sem-wait per instruction;
    hoist extras onto InstEventSemaphore instructions inserted just before."""
    import concourse.mybir as mybir
    n = 0
    for fn in nc.m.functions:
        for bb in fn.blocks:
            out = []
            for inst in bb.instructions:
                si = inst.sync_info
                if si is not None and si.on_wait and len(si.on_wait) > max_waits:
                    waits = list(si.on_wait)
                    excess, keep = waits[:-max_waits], waits[-max_waits:]
                    for i in range(0, len(excess), max_waits):
                        ev = mybir.InstEventSemaphore(
                            name=f"{inst.name}-wsplit{n}",
                            engine=inst.engine,
                            ins=[], outs=[],
                            sync_info=mybir.SyncInfo(
                                on_wait=list(excess[i:i + max_waits]),
                                on_update=[]),
                        )
                        n += 1
                        out.append(ev)
                    si.on_wait = keep
                out.append(inst)
            bb.instructions = out
    return n


def _build_program(debug=False):
    import concourse.bass as bass
    import concourse.mybir as mybir
    import concourse.tile as tile
    from concourse.bass import ts

    f32 = mybir.dt.float32
    f32r = mybir.dt.float32r
    bf16 = mybir.dt.bfloat16
    Exp = mybir.ActivationFunctionType.Exp

    nc = bass.Bass(num_devices=N_CORES)
    dbg = {}
    if debug:
        dbg["qT"] = nc.dram_tensor("dbg_qT", [128, ROWS], bf16,
                                   kind="ExternalOutput")
        dbg["kT"] = nc.dram_tensor("dbg_kT", [128, ROWS], bf16,
                                   kind="ExternalOutput")
        dbg["v"] = nc.dram_tensor("dbg_v", [128, 32, 2, HEAD + 1], bf16,
                                  kind="ExternalOutput")
        dbg["ctx0"] = nc.dram_tensor("dbg_ctx0", [64, ROWS], bf16,
                                     kind="ExternalOutput")
        dbg["ctx1"] = nc.dram_tensor("dbg_ctx1", [64, ROWS], bf16,
                                     kind="ExternalOutput")

    xT = nc.dram_tensor("xT", [D, ROWS], bf16, kind="ExternalInput")
    wqT = nc.dram_tensor("wqT", [D, 128], bf16, kind="ExternalInput")
    wkT = nc.dram_tensor("wkT", [D, 128], bf16, kind="ExternalInput")
    wvT = nc.dram_tensor("wvT", [D, 128], bf16, kind="ExternalInput")
    woT = nc.dram_tensor("woT", [D, D], bf16, kind="ExternalInput")
    bo = nc.dram_tensor("bo", [D], f32, kind="ExternalInput")
    masksq = nc.dram_tensor("masksq", [128, 128], bf16, kind="ExternalInput")
    sel = nc.dram_tensor("sel", [8, 8 * HEAD], f32r, kind="ExternalInput")
    out_rows = nc.dram_tensor("out_rows", [ROWS_PER_CORE, D], f32,
                              kind="ExternalOutput")

    with tile.TileContext(nc) as tc:
        with (
            tc.tile_pool(name="persist", bufs=1) as persist,
            tc.tile_pool(name="cp", bufs=4) as cp,
            tc.tile_pool(name="attn", bufs=6) as attn_pool,
            tc.tile_pool(name="nrm", bufs=4) as nrm_pool,
            tc.tile_pool(name="ps_work", bufs=4, space="PSUM") as ps_work,
            tc.tile_pool(name="ps_scores", bufs=2, space="PSUM") as ps_scores,
            tc.tile_pool(name="dram", bufs=1, space="DRAM") as dram,
        ):
            # ---- persistent SBUF state ----
            xT_sb = persist.tile([128, 8, ROWS], bf16)        # 64 KB/part
            wq_sb = persist.tile([128, 8, 128], bf16)
            wk_sb = persist.tile([128, 8, 128], bf16)
            wv_sb = persist.tile([128, 8, 128], bf16)
            woT_sb = persist.tile([128, 8, D], bf16)          # 16 KB/part
            qT_sb = persist.tile([128, ROWS], bf16)           # 8 KB/part
            kT_sb = persist.tile([128, ROWS], bf16)
            vT_sb = persist.tile([128, ROWS], bf16)
            # one tile per (rowtile, head): xbar-transpose needs offset-0
            # contiguous dst; col 64 is the ones column for the denominator
            v_tiles = [[persist.tile([128, HEAD + 1], bf16, tag=f"v{rt}_{h}",
                                     name=f"v{rt}_{h}")
                        for h in range(2)] for rt in range(32)]
            ctx_sb = [persist.tile([64, ROWS], bf16, tag=f"ctx{h}",
                                   name=f"ctx{h}")
                      for h in range(2)]
            mask_sb = persist.tile([128, 128], bf16)
            sel_sb = persist.tile([8, 8 * HEAD], f32r)
            den_all = [persist.tile([8, QT], f32, tag=f"den{b}",
                                    name=f"den{b}") for b in range(B)]
            den_rec = [persist.tile([8, QT], f32r, tag=f"rec{b}",
                                    name=f"rec{b}") for b in range(B)]
            bo_sb = persist.tile([128, D], f32)
            a2a_sb = [persist.tile([128, 8, RH], bf16, tag=f"a2a{b}",
                                   name=f"a2a{b}") for b in range(B)]

            # ---- loads: small weights first, xT split across both queues ----
            nc.sync.dma_start(wv_sb[:], wvT.rearrange("(t p) m -> p t m", p=128))
            nc.scalar.dma_start(wq_sb[:], wqT.rearrange("(t p) m -> p t m", p=128))
            nc.scalar.dma_start(wk_sb[:], wkT.rearrange("(t p) m -> p t m", p=128))
            for kt in range(8):
                eng = nc.sync if kt % 2 == 0 else nc.scalar
                eng.dma_start(xT_sb[:, kt, :], xT[ts(kt, 128), :])
            nc.gpsimd.dma_start(woT_sb[:], woT.rearrange("(t p) o -> p t o", p=128))
            nc.gpsimd.dma_start(mask_sb[:], masksq[:])
            nc.gpsimd.dma_start(sel_sb[:], sel[:])
            nc.gpsimd.dma_start(
                bo_sb[:], bass.AP(tensor=bo, offset=0,
                                  ap=[[0, 128], [1, D]]))
            for rt in range(32):
                for h in range(2):
                    nc.gpsimd.memset(v_tiles[rt][h][:, HEAD:HEAD + 1], 1.0)

            # ---- projections: vT first so transposes overlap q/k ----
            tp_i = 0
            for w_sb, dst in ((wv_sb, vT_sb), (wq_sb, qT_sb), (wk_sb, kT_sb)):
                for rt in range(8):
                    ps = ps_work.tile([128, 512], f32, tag="work")
                    for kt in range(8):
                        nc.tensor.matmul(ps, w_sb[:, kt, :],
                                         xT_sb[:, kt, ts(rt, 512)],
                                         start=(kt == 0), stop=(kt == 7))
                    nc.vector.tensor_copy(dst[:, ts(rt, 512)], ps)
                    if dst is vT_sb:
                        # the 4 row-tiles of 128 this 512-chunk covers
                        for rt128 in range(rt * 4, rt * 4 + 4):
                            for h in range(2):
                                eng = nc.sync if tp_i % 2 == 0 else nc.scalar
                                eng.dma_start_transpose(
                                    v_tiles[rt128][h][:, 0:HEAD],
                                    vT_sb[h * HEAD:(h + 1) * HEAD,
                                          ts(rt128, 128)])
                                tp_i += 1

            def attention_batch(b):
                units = []
                for qt in range(N_QT):
                    q0 = b * S + qt * QT
                    n_k = 4 * qt + 4
                    ps_av = [ps_work.tile([HEAD + 1, QT], f32, tag="work",
                                          name=f"av{b}_{qt}_{h}")
                             for h in range(2)]
                    for jk in range(n_k):
                        o = jk - 4 * qt       # >=0 on the diagonal band
                        c0 = max(o, 0) * 128  # first live query column
                        k0 = b * S + jk * KT
                        ps_s = ps_scores.tile([128, 2, QT], f32, tag="sc")
                        at = attn_pool.tile([128, 2, QT], bf16, tag="at")
                        for h in range(2):
                            hs = slice(h * HEAD, (h + 1) * HEAD)
                            nc.tensor.matmul(
                                ps_s[:, h, c0:QT],
                                kT_sb[hs, k0:k0 + KT],
                                qT_sb[hs, q0 + c0:q0 + QT],
                                start=True, stop=True)
                        nc.scalar.activation(at[:, :, c0:QT], ps_s[:, :, c0:QT],
                                             Exp, scale=INV_SCALE)
                        if o >= 0:
                            # partial causal sub-block: cols [c0, c0+128)
                            nc.vector.tensor_mul(
                                at[:, :, c0:c0 + 128],
                                at[:, :, c0:c0 + 128],
                                mask_sb[:, None, :].to_broadcast([128, 2, 128]))
                        for h in range(2):
                            nc.tensor.matmul(
                                ps_av[h][:, c0:QT],
                                v_tiles[b * 16 + jk][h][:],
                                at[:, h, c0:QT],
                                start=(jk == 0), stop=(jk == n_k - 1))
                    for h in range(2):
                        u = qt * 2 + h
                        nc.vector.tensor_copy(ctx_sb[h][:, q0:q0 + QT],
                                              ps_av[h][0:HEAD, :])
                        dtmp = nrm_pool.tile([HEAD + 1, QT], f32, tag="dtmp")
                        nc.vector.tensor_copy(dtmp[HEAD:HEAD + 1, :],
                                              ps_av[h][HEAD:HEAD + 1, :])
                        nc.sync.dma_start(den_all[b][u:u + 1, :],
                                          dtmp[HEAD:HEAD + 1, :])
                        units.append((h, q0, u))
                return units

            def norm_and_a2a(b, units):
                with nc.allow_low_precision(
                        reason="softmax denominators: f32r keeps ~19 mantissa "
                               "bits, ample for a 1/x broadcast"):
                    nc.vector.reciprocal(den_rec[b][:], den_all[b][:])
                for (h, q0, u) in units:
                    ps_b = ps_scores.tile([HEAD, QT], f32, tag="sc")
                    nc.tensor.matmul(ps_b,
                                     sel_sb[:, ts(u, HEAD)],
                                     den_rec[b][:],
                                     start=True, stop=True)
                    nc.vector.tensor_mul(ctx_sb[h][:, q0:q0 + QT],
                                         ctx_sb[h][:, q0:q0 + QT],
                                         ps_b[:])
                # A2A for this batch: shard s = rows [b*2048 + s*256, +256)
                a2a_in = dram.tile([8, 128, RH], bf16, tag=f"a2ain{b}",
                                   name=f"a2ain{b}")
                a2a_out = dram.tile([8, 128, RH], bf16, tag=f"a2aout{b}",
                                    name=f"a2aout{b}")
                for h in range(2):
                    nc.sync.dma_start(
                        a2a_in[:, h * 64:(h + 1) * 64, :]
                        .rearrange("s p r -> p s r"),
                        ctx_sb[h][:, b * S:(b + 1) * S]
                        .rearrange("p (s r) -> p s r", s=8))
                nc.gpsimd.collective_compute(
                    "AllToAll", mybir.AluOpType.bypass,
                    replica_groups=[list(range(N_CORES))],
                    ins=[a2a_in[:].opt()], outs=[a2a_out[:].opt()])
                nc.scalar.dma_start(a2a_sb[b][:],
                                    a2a_out[:].rearrange("t p r -> p t r"))

            def outproj(b):
                # this core's rows for batch b: out_rows[b*256 : (b+1)*256]
                for rt in range(2):
                    for nh in range(2):
                        ps = ps_work.tile([128, 512], f32, tag="work")
                        for t in range(8):
                            nc.tensor.matmul(ps,
                                             a2a_sb[b][:, t, ts(rt, 128)],
                                             woT_sb[:, t, ts(nh, 512)],
                                             start=(t == 0), stop=(t == 7))
                        ob = cp.tile([128, 512], f32, tag="ob")
                        nc.vector.tensor_add(ob, ps, bo_sb[:, ts(nh, 512)])
                        nc.sync.dma_start(
                            out_rows[ts(b * 2 + rt, 128), ts(nh, 512)], ob)

            units0 = attention_batch(0)
            norm_and_a2a(0, units0)
            units1 = attention_batch(1)
            outproj(0)
            norm_and_a2a(1, units1)
            outproj(1)

            if debug:
                nc.sync.dma_start(dbg["qT"][:], qT_sb[:])
                nc.sync.dma_start(dbg["kT"][:], kT_sb[:])
                for rt in range(32):
                    for h in range(2):
                        nc.sync.dma_start(dbg["v"][:, rt, h, :],
                                          v_tiles[rt][h][:])
                nc.sync.dma_start(dbg["ctx0"][:], ctx_sb[0][:])
                nc.sync.dma_start(dbg["ctx1"][:], ctx_sb[1][:])

    _split_excess_waits(nc)
    return nc


def _make_masksq():
    p = np.arange(128)[:, None]
    r = np.arange(128)[None, :]
    return (p <= r).astype(BF16)


def _make_sel():
    # sel[k, u*64+m] = 1 if k == u : broadcasts den_rec row u over 64 partitions
    s = np.zeros((8, 8 * HEAD), np.float32)
    for u in range(8):
        s[u, u * HEAD:(u + 1) * HEAD] = 1.0
    return s


def _shard_inputs(x, Wq, Wk, Wv, Wo, bo):
    xT = np.ascontiguousarray(
        x.reshape(ROWS, D).T).astype(BF16)            # [D, 4096]
    woT = np.ascontiguousarray(Wo.T).astype(BF16)     # [D, D]
    masksq = _make_masksq()
    sel = _make_sel()
    bo32 = np.ascontiguousarray(bo.astype(np.float32))
    maps = []
    for c in range(N_CORES):
        rs = slice(c * 128, (c + 1) * 128)
        maps.append({
            "xT": xT,
            "wqT": np.ascontiguousarray(Wq[rs].T).astype(BF16),
            "wkT": np.ascontiguousarray(Wk[rs].T).astype(BF16),
            "wvT": np.ascontiguousarray(Wv[rs].T).astype(BF16),
            "woT": woT,
            "bo": bo32,
            "masksq": masksq,
            "sel": sel,
        })
    return maps


def kernel(x, Wq, Wk, Wv, Wo, bo, trace=False):
    global _compiled
    _install_axon_profile_shim()
    from concourse.bass_utils import run_bass_kernel_spmd

    x = np.asarray(x, dtype=np.float32)
    Wq = np.asarray(Wq, dtype=np.float32)
    Wk = np.asarray(Wk, dtype=np.float32)
    Wv = np.asarray(Wv, dtype=np.float32)
    Wo = np.asarray(Wo, dtype=np.float32)
    bo = np.asarray(bo, dtype=np.float32)

    if _compiled is None:
        _compiled = _build_program()
    nc = _compiled

    in_maps = _shard_inputs(x, Wq, Wk, Wv, Wo, bo)
    res = run_bass_kernel_spmd(nc, in_maps, core_ids=list(range(N_CORES)),
                               trace=trace)
    out = np.empty((ROWS, D), np.float32)
    for c in range(N_CORES):
        r = res.results[c]["out_rows"]
        out[c * RH:(c + 1) * RH] = r[0:RH]
        out[S + c * RH:S + (c + 1) * RH] = r[RH:2 * RH]
    out = out.reshape(B, S, D)
    if trace:
        kernel.last_exec_time_ns = res.exec_time_ns
        kernel.last_results = res
    return out
